# revision 7
# baseline (speedup 1.0000x reference)
"""Trainium2 Bass kernel for Mixtral-style attention (GQA + NeoX RoPE + causal).

Tensor-parallel over heads across 8 NeuronCores: each core owns 4 query heads
and their shared KV head (GQA group intact). Wqkv is column-sharded, Wo is
row-sharded; per-core fp32 partial outputs are summed on the host.

Per-core dataflow (feature-major layouts; all matmuls bf16 with fp32 PSUM):
  1. QKV projection: qkvT[j, t] = sum_h Wqkv[h, j] * hidden[t, h]
     (stationary = Wqkv tile, moving = hiddenT tile streamed from DRAM)
  2. NeoX RoPE applied to q/k straight out of PSUM (DVE), keeping [d, t] layout
  3. Attention per head with transposed scores: sT[s, t] = k . q so softmax's
     sum runs over the partition dim via a ones-vector matmul; exp on ACT
     without max subtraction (scores are small for this problem size); causal
     masking via GpSimd affine_select zeroing on diagonal-block tiles, with
     fully-masked leading columns of diagonal tiles skipped entirely
  4. PV: attnT[d, t] += v[s, d].T @ P[s, t] with v transposed once via PE
  5. Normalization by 1/den broadcast across partitions on GpSimd
  6. Output projection row-shard: outT_partial[o, t] accumulated over the
     core's 512 features, written fp32 to DRAM.

SBUF/PSUM pools are scoped so phase 1 (QKV) and phase 2 (attention) can
overlap in the scheduler, while phase 3 reuses their space.
"""

import numpy as np
import ml_dtypes
from contextlib import ExitStack

import concourse.bass as bass
import concourse.tile as tile
from concourse import bacc, mybir
from concourse.bass import ts
from concourse.bass_utils import run_bass_kernel_spmd
from concourse.masks import make_identity

BF16 = mybir.dt.bfloat16
F32 = mybir.dt.float32
F16 = mybir.dt.float16
AF = mybir.ActivationFunctionType

T = 2048
HID = 4096
NH = 32
NKV = 8
D = 128
NCORES = 8
NHL = NH // NCORES            # 4 query heads per core
HO = HID // 128               # 32 hidden-dim k-tiles
TCH = 512
NTC = T // TCH                # 4 t-chunks
NSB = T // 128                # 16 s-blocks
SCALING = float(D) ** -0.5
ROPE_THETA = 1000000.0


def build_kernel():
    nc = bacc.Bacc("TRN2", target_bir_lowering=False, debug=False, num_devices=NCORES)

    hT = nc.dram_tensor("hT", [HID, T], BF16, kind="ExternalInput")
    wqkv = nc.dram_tensor("wqkv", [6, 128, HO, 128], BF16, kind="ExternalInput")
    wo = nc.dram_tensor("wo", [128, NHL, HID], BF16, kind="ExternalInput")
    cosT = nc.dram_tensor("cosT", [64, T], F16, kind="ExternalInput")
    sinT = nc.dram_tensor("sinT", [64, T], F16, kind="ExternalInput")
    outT = nc.dram_tensor("outT", [HID, T], F32, kind="ExternalOutput")

    with tile.TileContext(nc) as tc, ExitStack() as ctx:
        # ---- pools that live through phases 1+2 (and some through 3) ----
        per = ctx.enter_context(tc.tile_pool(name="per", bufs=1))
        qT_t = per.tile([128, NHL, T], BF16, tag="qT")
        kT_t = per.tile([128, T], BF16, tag="kT")
        vnat_t = per.tile([128, NSB, 128], BF16, tag="vnat")
        attnT_t = per.tile([128, NHL, T], BF16, tag="attnT")
        ones_s = per.tile([128, 1], BF16, tag="ones_s")
        nc.vector.memset(ones_s, 1.0)

        ppool = ctx.enter_context(tc.tile_pool(name="ppool", bufs=3))
        rdenp = ctx.enter_context(tc.tile_pool(name="rdenp", bufs=1))
        repp = ctx.enter_context(tc.tile_pool(name="repp", bufs=1))

        with ExitStack() as s12:
            # attention PSUM pools open before phase-1's so both coexist
            sc_ps = s12.enter_context(tc.tile_pool(name="sc_ps", bufs=2, space="PSUM"))
            att_ps_pool = s12.enter_context(tc.tile_pool(name="att_ps", bufs=2, space="PSUM"))
            den_ps_pool = s12.enter_context(tc.tile_pool(name="den_ps", bufs=2, space="PSUM"))

            # ================= phase 1: QKV projection =================
            with ExitStack() as s1:
                p1per = s1.enter_context(tc.tile_pool(name="p1per", bufs=1))
                hpool = s1.enter_context(tc.tile_pool(name="hpool", bufs=HO))
                wqpool = s1.enter_context(tc.tile_pool(name="wqpool", bufs=2))
                ropetmp = s1.enter_context(tc.tile_pool(name="ropetmp", bufs=2))
                qkv_ps = s1.enter_context(tc.tile_pool(name="qkv_ps", bufs=2, space="PSUM"))

                # first j-pass weights before the bulk hT stream
                wq_first = wqpool.tile([128, HO, 128], BF16, tag="wq")
                nc.sync.dma_start(out=wq_first, in_=wqkv.ap()[4])

                cos_t = p1per.tile([64, T], F16, tag="cos")
                sin_t = p1per.tile([64, T], F16, tag="sin")
                vT_t = p1per.tile([128, T], BF16, tag="vT")
                ident_t = p1per.tile([128, 128], BF16, tag="ident")
                nc.sync.dma_start(out=cos_t, in_=cosT.ap())
                nc.sync.dma_start(out=sin_t, in_=sinT.ap())
                make_identity(nc, ident_t)

                ht = []
                hT_r = hT.ap().rearrange("(ho p) t -> p ho t", p=128)
                for ho in range(HO):
                    t_ = hpool.tile([128, T], BF16, tag="ht")
                    nc.sync.dma_start(out=t_, in_=hT_r[:, ho, :])
                    ht.append(t_)

                # j blocks: 0-3 = q heads, 4 = k, 5 = v (k, v first)
                for j in (4, 5, 0, 1, 2, 3):
                    if j == 4:
                        wq_sb = wq_first
                    else:
                        wq_sb = wqpool.tile([128, HO, 128], BF16, tag="wq")
                        nc.sync.dma_start(out=wq_sb, in_=wqkv.ap()[j])
                    for c in range(NTC):
                        ps = qkv_ps.tile([128, TCH], F32, tag="qkvps")
                        for ho in range(HO):
                            nc.tensor.matmul(
                                ps, wq_sb[:, ho, :], ht[ho][:, ts(c, TCH)],
                                start=(ho == 0), stop=(ho == HO - 1),
                            )
                        if j == 5:
                            nc.scalar.copy(out=vT_t[:, ts(c, TCH)], in_=ps)
                        else:
                            dst = kT_t if j == 4 else qT_t[:, j, :]
                            co = cos_t[:, ts(c, TCH)]
                            si = sin_t[:, ts(c, TCH)]
                            x1 = ps[0:64, :]
                            x2 = ps[64:128, :]
                            # walrus requires SBUF-SBUF operand pairs to share
                            # a start partition; PSUM-SBUF pairs may differ.
                            tmp = ropetmp.tile([128, TCH], F32, tag="rt")
                            tmp2 = ropetmp.tile([128, TCH], F32, tag="rt")
                            nc.vector.tensor_mul(tmp[0:64, :], x1, co)
                            nc.vector.tensor_mul(tmp2[0:64, :], x2, si)
                            nc.vector.tensor_sub(
                                dst[0:64, ts(c, TCH)], tmp[0:64, :], tmp2[0:64, :]
                            )
                            nc.vector.tensor_mul(tmp[64:128, :], x2, co)
                            nc.vector.tensor_mul(tmp2[64:128, :], x1, si)
                            nc.vector.tensor_add(
                                dst[64:128, ts(c, TCH)], tmp[64:128, :], tmp2[64:128, :]
                            )
                    if j == 5:
                        # transpose v to natural [s, d] layout (PE, via identity)
                        for sb in range(NSB):
                            tp = qkv_ps.tile([128, 128], BF16, tag="qkvps")
                            nc.tensor.transpose(tp, vT_t[:, ts(sb, 128)], ident_t)
                            nc.scalar.copy(out=vnat_t[:, sb, :], in_=tp)

            # ================= phase 2: attention =================
            for h in range(NHL):
                for c in range(NTC):
                    nblk = 4 * (c + 1)
                    att_ps = att_ps_pool.tile([128, TCH], F32, tag="att")
                    den_ps = den_ps_pool.tile([1, TCH], F32, tag="den")
                    for sb in range(nblk):
                        r = sb - 4 * c
                        off = 128 * r if r > 0 else 0
                        w = TCH - off
                        scp = sc_ps.tile([128, TCH], F32, tag="sc")
                        nc.tensor.matmul(
                            scp[:, off:], kT_t[:, ts(sb, 128)],
                            qT_t[:, h, c * TCH + off: (c + 1) * TCH],
                            start=True, stop=True,
                        )
                        p_sb = ppool.tile([128, TCH], BF16, tag="p")
                        nc.scalar.activation(
                            p_sb[:, off:], scp[:, off:], AF.Exp, scale=SCALING
                        )
                        if r >= 0:
                            # zero entries with t < s inside the diagonal block
                            nc.gpsimd.affine_select(
                                out=p_sb[:, off:], in_=p_sb[:, off:],
                                compare_op=mybir.AluOpType.is_ge,
                                fill=0.0, base=0,
                                pattern=[[1, w]], channel_multiplier=-1,
                            )
                        nc.tensor.matmul(
                            att_ps[:, off:], vnat_t[:, sb, :], p_sb[:, off:],
                            start=(sb == 0), stop=(sb == nblk - 1),
                        )
                        nc.tensor.matmul(
                            den_ps[:, off:], ones_s, p_sb[:, off:],
                            start=(sb == 0), stop=(sb == nblk - 1),
                        )
                    rden = rdenp.tile([1, TCH], F32, tag="rden")
                    nc.vector.reciprocal(rden, den_ps)
                    rep_sb = repp.tile([128, TCH], F32, tag="rep")
                    nc.gpsimd.partition_broadcast(rep_sb, rden)
                    nc.vector.tensor_mul(
                        attnT_t[:, h, ts(c, TCH)], att_ps, rep_sb
                    )

        # ================= phase 3: output projection =================
        wopool = ctx.enter_context(tc.tile_pool(name="wopool", bufs=1))
        opool = ctx.enter_context(tc.tile_pool(name="opool", bufs=3))
        out_ps_pool = ctx.enter_context(tc.tile_pool(name="out_ps", bufs=2, space="PSUM"))

        wo_sb = wopool.tile([128, NHL, HID], BF16, tag="wo")
        nc.sync.dma_start(out=wo_sb, in_=wo.ap())
        for ot in range(HID // 128):
            for half in range(2):
                ps2 = out_ps_pool.tile([128, 2, TCH], F32, tag="ops")
                for df in range(NHL):
                    for t2 in range(2):
                        cc = half * 2 + t2
                        nc.tensor.matmul(
                            ps2[:, t2, :], wo_sb[:, df, ts(ot, 128)],
                            attnT_t[:, df, ts(cc, TCH)],
                            start=(df == 0), stop=(df == NHL - 1),
                        )
                o_sb = opool.tile([128, 2 * TCH], F32, tag="osb")
                if (ot * 2 + half) % 2 == 0:
                    nc.vector.tensor_copy(o_sb, ps2.rearrange("p a b -> p (a b)"))
                else:
                    nc.scalar.copy(out=o_sb, in_=ps2.rearrange("p a b -> p (a b)"))
                nc.sync.dma_start(
                    out=outT.ap()[ts(ot, 128), ts(half, 2 * TCH)], in_=o_sb
                )

    nc.compile()
    return nc


_CACHE = {}


def _get_nc():
    if "nc" not in _CACHE:
        _CACHE["nc"] = build_kernel()
    return _CACHE["nc"]


def make_inputs(positions, hidden_states, Wqkv, Wo):
    """Host-side shard prep. Returns per-core input maps."""
    bf = ml_dtypes.bfloat16
    positions = np.asarray(positions)
    hidden_states = np.asarray(hidden_states, dtype=np.float32)
    Wqkv = np.asarray(Wqkv, dtype=np.float32)
    Wo = np.asarray(Wo, dtype=np.float32)

    hT = np.ascontiguousarray(hidden_states.astype(bf).T)  # [HID, T]

    half = D // 2
    inv_freq = (
        1.0 / (np.float32(ROPE_THETA) ** (np.arange(0, half, dtype=np.float32) / np.float32(half)))
    ).astype(np.float32)
    freqs = positions.astype(np.float32)[:, None] * inv_freq[None, :]  # [T, 64]
    cosT = np.ascontiguousarray(np.cos(freqs).astype(np.float16).T)
    sinT = np.ascontiguousarray(np.sin(freqs).astype(np.float16).T)

    q_size = NH * D
    kv_off = q_size + NKV * D
    in_maps = []
    for c in range(NCORES):
        qcols = Wqkv[:, 512 * c: 512 * (c + 1)]
        kcol = Wqkv[:, q_size + 128 * c: q_size + 128 * (c + 1)]
        vcol = Wqkv[:, kv_off + 128 * c: kv_off + 128 * (c + 1)]
        shard = np.concatenate([qcols, kcol, vcol], axis=1).astype(bf)  # [HID, 768]
        wq_dev = np.ascontiguousarray(
            shard.reshape(HO, 128, 6, 128).transpose(2, 1, 0, 3)
        )  # [6, 128, HO, 128]
        wo_shard = Wo[512 * c: 512 * (c + 1), :].astype(bf)  # [512, HID]
        wo_dev = np.ascontiguousarray(
            wo_shard.reshape(NHL, 128, HID).transpose(1, 0, 2)
        )  # [128, NHL, HID]
        in_maps.append(
            {
                "hT": hT,
                "wqkv": wq_dev,
                "wo": wo_dev,
                "cosT": cosT,
                "sinT": sinT,
            }
        )
    return in_maps


def kernel(positions, hidden_states, Wqkv, Wo):
    in_maps = make_inputs(positions, hidden_states, Wqkv, Wo)
    res = run_bass_kernel_spmd(_get_nc(), in_maps, list(range(NCORES)))
    acc = res.results[0]["outT"].copy()
    for c in range(1, NCORES):
        acc += res.results[c]["outT"]
    return np.ascontiguousarray(acc.T)


# revision 19
# speedup vs baseline: 3.0602x; 3.0602x over previous
"""Trainium2 Bass kernel for Mixtral-style attention (GQA + NeoX RoPE + causal).

Tensor-parallel over heads across 8 NeuronCores: each core owns 4 query heads
and their shared KV head (GQA group intact). Wqkv is column-sharded, Wo is
row-sharded; per-core fp32 partial outputs are summed on the host.

Per-core dataflow (feature-major layouts; all matmuls bf16 with fp32 PSUM):
  1. QKV projection: qkvT[j, t] = sum_h Wqkv[h, j] * hidden[t, h]
     (stationary = Wqkv tile, moving = hiddenT tile streamed from DRAM)
  2. NeoX RoPE applied to q/k straight out of PSUM (DVE), keeping [d, t] layout
  3. Attention per head with transposed scores: sT[s, t] = k . q so softmax's
     sum runs over the partition dim via a ones-vector matmul; exp on ACT
     without max subtraction (scores are small for this problem size); causal
     masking via GpSimd affine_select zeroing on diagonal-block tiles, with
     fully-masked leading columns of diagonal tiles skipped entirely
  4. PV: attnT[d, t] += v[s, d].T @ P[s, t] with v transposed once via PE
  5. Normalization by 1/den broadcast across partitions on GpSimd
  6. Output projection row-shard: outT_partial[o, t] accumulated over the
     core's 512 features, written fp32 to DRAM.

SBUF/PSUM pools are scoped so phase 1 (QKV) and phase 2 (attention) can
overlap in the scheduler, while phase 3 reuses their space.
"""

import numpy as np
import ml_dtypes
from contextlib import ExitStack

import concourse.bass as bass
import concourse.tile as tile
from concourse import bacc, mybir
from concourse.bass import ts
from concourse.bass_utils import run_bass_kernel_spmd
from concourse.masks import make_identity

BF16 = mybir.dt.bfloat16
F32 = mybir.dt.float32
F16 = mybir.dt.float16
AF = mybir.ActivationFunctionType

T = 2048
HID = 4096
NH = 32
NKV = 8
D = 128
NCORES = 8
NHL = NH // NCORES            # 4 query heads per core
HO = HID // 128               # 32 hidden-dim k-tiles
TCH = 512
NTC = T // TCH                # 4 t-chunks
NSB = T // 128                # 16 s-blocks
SCALING = float(D) ** -0.5
ROPE_THETA = 1000000.0


def build_kernel():
    nc = bacc.Bacc("TRN2", target_bir_lowering=False, debug=False, num_devices=NCORES)

    hT = nc.dram_tensor("hT", [HID, T], BF16, kind="ExternalInput")
    wqkv = nc.dram_tensor("wqkv", [6, 128, HO, 128], BF16, kind="ExternalInput")
    wo = nc.dram_tensor("wo", [128, NHL, HID], BF16, kind="ExternalInput")
    cosT = nc.dram_tensor("cosT", [128, T], F16, kind="ExternalInput")
    sinT = nc.dram_tensor("sinT", [128, T], F16, kind="ExternalInput")
    outT = nc.dram_tensor("outT", [HID, T], F32, kind="ExternalOutput")

    with tile.TileContext(nc) as tc, ExitStack() as ctx:
        # ---- pools that live through phases 1+2 (and some through 3) ----
        per = ctx.enter_context(tc.tile_pool(name="per", bufs=1))
        qT_t = per.tile([128, NHL, T], BF16, tag="qT")
        kT_t = per.tile([128, T], BF16, tag="kT")
        vnat_t = per.tile([128, NSB, 128], BF16, tag="vnat")
        ones_s = per.tile([128, 1], BF16, tag="ones_s")
        nc.vector.memset(ones_s, 1.0)

        # SBUF pools for phase 1 (closed before phase 3 opens its pools)
        s1 = ExitStack()
        p1per = s1.enter_context(tc.tile_pool(name="p1per", bufs=1))
        hpool = s1.enter_context(tc.tile_pool(name="hpool", bufs=HO))
        wqpool = s1.enter_context(tc.tile_pool(name="wqpool", bufs=4))
        ropetmp = s1.enter_context(tc.tile_pool(name="ropetmp", bufs=2))

        def rope(dst, c, ps):
            # NeoX rotation with full-width ops: cos_t rows are [cos; cos] and
            # sin_t rows are [-sin; +sin], so with swp = [x2; x1] (partition
            # swap via two ACT copies, PSUM->SBUF so mixed start partitions
            # are walrus-legal):
            #   dst = ps * cos_t + swp * sin_t
            swp = ropetmp.tile([128, TCH], F16, tag="swp")
            nc.scalar.copy(out=swp[0:64, :], in_=ps[64:128, :])
            nc.scalar.copy(out=swp[64:128, :], in_=ps[0:64, :])
            t1 = ropetmp.tile([128, TCH], F16, tag="rt")
            t2 = ropetmp.tile([128, TCH], F16, tag="rt")
            nc.vector.tensor_mul(t1, ps, cos_t[:, ts(c, TCH)])
            nc.vector.tensor_mul(t2, swp, sin_t[:, ts(c, TCH)])
            nc.vector.tensor_add(dst[:, ts(c, TCH)], t1, t2)

        # ====== phase 1a: k+v projections interleaved with the hT stream ======
        # One 8-bank PSUM tile holds all (j in {k,v}) x (4 t-chunks)
        # accumulators so the ho loop is outermost and PE tracks the DMA.
        sA = ExitStack()
        qkvA_ps = sA.enter_context(tc.tile_pool(name="qkvA_ps", bufs=1, space="PSUM"))

        # One DMA per tile: each dma_start costs ~625ns of serialized HWDGE
        # issue regardless of size, so fewer+bigger transfers win. Order is
        # need-order: wq4/ht0/wq5 first, then the hT stream, wq0/wq1 later.
        wq_r = wqkv.ap()
        hT_r = hT.ap().rearrange("(ho p) t -> p ho t", p=128)
        ht = [hpool.tile([128, T], BF16, tag="ht", name=f"ht{i}") for i in range(HO)]
        HH = HO // 2
        # each j's weights as two half tiles for finer DMA dependencies
        wq_tiles = {}
        for j in (4, 5, 0, 1):
            wq_tiles[j] = [
                wqpool.tile([128, HH, 128], BF16, tag="wq", name=f"wq{j}{p}")
                for p in range(2)
            ]

        def wq_slice(j, ho):
            return wq_tiles[j][ho // HH][:, ho % HH, :]

        nc.sync.dma_start(out=ht[0], in_=hT_r[:, 0, :])
        nc.sync.dma_start(out=wq_tiles[4][0], in_=wq_r[4][:, :HH, :])
        nc.sync.dma_start(out=wq_tiles[5][0], in_=wq_r[5][:, :HH, :])
        nc.sync.dma_start(out=wq_tiles[4][1], in_=wq_r[4][:, HH:, :])
        nc.sync.dma_start(out=wq_tiles[5][1], in_=wq_r[5][:, HH:, :])

        cos_t = p1per.tile([128, T], F16, tag="cos")
        sin_t = p1per.tile([128, T], F16, tag="sin")
        vT_t = [
            p1per.tile([128, TCH], BF16, tag=f"vT{c}", name=f"vT{c}")
            for c in range(NTC)
        ]
        ident_t = p1per.tile([128, 128], BF16, tag="ident")

        for ho in range(1, HO):
            nc.sync.dma_start(out=ht[ho], in_=hT_r[:, ho, :])
        nc.sync.dma_start(out=cos_t, in_=cosT.ap())
        nc.sync.dma_start(out=sin_t, in_=sinT.ap())
        make_identity(nc, ident_t)
        for p in range(2):
            nc.sync.dma_start(out=wq_tiles[0][p], in_=wq_r[0][:, p * HH:(p + 1) * HH, :])
            nc.sync.dma_start(out=wq_tiles[1][p], in_=wq_r[1][:, p * HH:(p + 1) * HH, :])

        # 8 independent 1-bank accumulators; ho-outer while the hT stream is
        # in flight, then per-(jj,c) tails so drains stagger
        psA = [qkvA_ps.tile([128, TCH], F32, tag=f"psA{i}", name=f"psA{i}") for i in range(8)]
        HSPLIT = 24
        for ho in range(HSPLIT):
            for jj in (4, 5):
                for c in range(NTC):
                    g = (jj - 4) * NTC + c
                    nc.tensor.matmul(
                        psA[g], wq_slice(jj, ho), ht[ho][:, ts(c, TCH)],
                        start=(ho == 0), stop=False,
                    )
        for jj in (4, 5):
            for c in range(NTC):
                g = (jj - 4) * NTC + c
                for ho in range(HSPLIT, HO):
                    nc.tensor.matmul(
                        psA[g], wq_slice(jj, ho), ht[ho][:, ts(c, TCH)],
                        start=False, stop=(ho == HO - 1),
                    )
                if jj == 4:
                    rope(kT_t, c, psA[g])
                else:
                    nc.scalar.copy(out=vT_t[c], in_=psA[g])


        for j in (0, 1, 2, 3):
            if j not in wq_tiles:
                wq_tiles[j] = [
                    wqpool.tile([128, HH, 128], BF16, tag="wq", name=f"wq{j}{p}")
                    for p in range(2)
                ]
                for p in range(2):
                    nc.sync.dma_start(
                        out=wq_tiles[j][p], in_=wq_r[j][:, p * HH:(p + 1) * HH, :]
                    )
            for c in range(NTC):
                # rotate through the 8 pair-pass accumulator slots so each
                # projection reuses the earliest-freed PSUM bank
                g = (j * NTC + c) % 8
                ps = qkvA_ps.tile([128, TCH], F32, tag=f"psA{g}", name=f"ps{j}{c}")
                for ho in range(HO):
                    nc.tensor.matmul(
                        ps, wq_slice(j, ho), ht[ho][:, ts(c, TCH)],
                        start=(ho == 0), stop=(ho == HO - 1),
                    )
                rope(qT_t[:, j, :], c, ps)

        # v transpose last: it fills the PE pipeline across the phase boundary
        for sb in range(NSB):
            tp = qkvA_ps.tile([128, 128], BF16, tag=f"psA{sb % 8}", name=f"tp{sb}")
            nc.tensor.transpose(tp, vT_t[sb // 4][:, ts(sb % 4, 128)], ident_t)
            nc.scalar.copy(out=vnat_t[:, sb, :], in_=tp)
        sA.close()
        s1.close()

        # ================= phase 2: attention =================
        p23per = ctx.enter_context(tc.tile_pool(name="p23per", bufs=1))
        attnT_t = p23per.tile([128, NHL, T], BF16, tag="attnT")
        mask_t = p23per.tile([128, 2, TCH], BF16, tag="mask")
        # mask[p, r, j] = 1.0 where j >= 128 r + p (diagonal block patterns)
        nc.gpsimd.memset(mask_t, 1.0)
        for r in range(2):
            nc.gpsimd.affine_select(
                out=mask_t[:, r, :], in_=mask_t[:, r, :],
                compare_op=mybir.AluOpType.is_ge,
                fill=0.0, base=-128 * r, pattern=[[1, TCH]], channel_multiplier=-1,
            )
        ppool = ctx.enter_context(tc.tile_pool(name="ppool", bufs=4))
        rdenp = ctx.enter_context(tc.tile_pool(name="rdenp", bufs=2))
        repp = ctx.enter_context(tc.tile_pool(name="repp", bufs=2))

        s2ps = ExitStack()
        sc_ps = s2ps.enter_context(tc.tile_pool(name="sc_ps", bufs=3, space="PSUM"))
        att_ps_pool = s2ps.enter_context(tc.tile_pool(name="att_ps", bufs=1, space="PSUM"))
        den_ps_pool = s2ps.enter_context(tc.tile_pool(name="den_ps", bufs=1, space="PSUM"))

        for h in range(NHL):
            for c in (3, 2, 1, 0):
                nblk = 4 * (c + 1)
                att_ps = att_ps_pool.tile([128, TCH], F32, tag="att")
                den_ps = den_ps_pool.tile([1, TCH], F32, tag="den")
                # full-width tiles (sb <= 4c, i.e. r <= 0) are paired so one
                # wide exp on ACT amortizes the per-op overhead; trimmed
                # diagonal tiles (r >= 1) go solo at reduced width
                groups = []
                i = 0
                while i <= 4 * c:
                    if i + 1 <= 4 * c:
                        groups.append((i, i + 1))
                        i += 2
                    else:
                        groups.append((i,))
                        i += 1
                for sb in range(4 * c + 1, nblk):
                    groups.append((sb,))
                for grp in groups:
                    scp = sc_ps.tile([128, 2, TCH], F32, tag="sc")
                    p_sb = ppool.tile([128, 2, TCH], BF16, tag="p")
                    offs = []
                    for i, sb in enumerate(grp):
                        r = sb - 4 * c
                        off = 128 * r if r > 0 else 0
                        offs.append((i, sb, r, off))
                        nc.tensor.matmul(
                            scp[:, i, off:], kT_t[:, ts(sb, 128)],
                            qT_t[:, h, c * TCH + off: (c + 1) * TCH],
                            start=True, stop=True,
                        )
                    if len(grp) == 2:
                        nc.scalar.activation(
                            p_sb.rearrange("p a b -> p (a b)"),
                            scp.rearrange("p a b -> p (a b)"),
                            AF.Exp, scale=SCALING,
                        )
                    else:
                        off0 = offs[0][3]
                        nc.scalar.activation(
                            p_sb[:, 0, off0:], scp[:, 0, off0:],
                            AF.Exp, scale=SCALING,
                        )
                    for i, sb, r, off in offs:
                        if r in (0, 1):
                            # diagonal blocks r0/r1: mask-multiply on DVE
                            nc.vector.tensor_mul(
                                p_sb[:, i, off:], p_sb[:, i, off:],
                                mask_t[:, r, off:],
                            )
                        elif r > 1:
                            # trimmed diagonal block: zero t < s on GpSimd
                            nc.gpsimd.affine_select(
                                out=p_sb[:, i, off:], in_=p_sb[:, i, off:],
                                compare_op=mybir.AluOpType.is_ge,
                                fill=0.0, base=0,
                                pattern=[[1, TCH - off]], channel_multiplier=-1,
                            )
                    for i, sb, r, off in offs:
                        nc.tensor.matmul(
                            att_ps[:, off:], vnat_t[:, sb, :], p_sb[:, i, off:],
                            start=(sb == 0), stop=(sb == nblk - 1),
                        )
                        nc.tensor.matmul(
                            den_ps[:, off:], ones_s, p_sb[:, i, off:],
                            start=(sb == 0), stop=(sb == nblk - 1),
                        )
                rden = rdenp.tile([1, TCH], F32, tag="rden")
                nc.vector.reciprocal(rden, den_ps)
                rep_sb = repp.tile([128, TCH], F32, tag="rep")
                nc.gpsimd.partition_broadcast(rep_sb, rden)
                # copy the unnormalized output out first (frees the PSUM bank
                # without waiting on the reciprocal/broadcast chain), then
                # normalize in place
                nc.vector.tensor_copy(attnT_t[:, h, ts(c, TCH)], att_ps)
                nc.vector.tensor_mul(
                    attnT_t[:, h, ts(c, TCH)], attnT_t[:, h, ts(c, TCH)], rep_sb
                )
        s2ps.close()

        # ================= phase 3: output projection =================
        wopool = ctx.enter_context(tc.tile_pool(name="wopool", bufs=1))
        opool = ctx.enter_context(tc.tile_pool(name="opool", bufs=3))
        out_ps_pool = ctx.enter_context(tc.tile_pool(name="out_ps", bufs=2, space="PSUM"))

        wo_sb = wopool.tile([128, NHL, HID], BF16, tag="wo")
        nc.sync.dma_start(out=wo_sb, in_=wo.ap())
        for ot in range(HID // 128):
            o_sb = opool.tile([128, T], F32, tag="osb")
            for half in range(2):
                ps2 = out_ps_pool.tile([128, 2, TCH], F32, tag="ops")
                for df in range(NHL):
                    for t2 in range(2):
                        cc = half * 2 + t2
                        nc.tensor.matmul(
                            ps2[:, t2, :], wo_sb[:, df, ts(ot, 128)],
                            attnT_t[:, df, ts(cc, TCH)],
                            start=(df == 0), stop=(df == NHL - 1),
                        )
                if half == 0:
                    nc.vector.tensor_copy(
                        o_sb[:, 0: 2 * TCH], ps2.rearrange("p a b -> p (a b)")
                    )
                else:
                    nc.scalar.copy(
                        out=o_sb[:, 2 * TCH: 4 * TCH],
                        in_=ps2.rearrange("p a b -> p (a b)"),
                    )
            nc.sync.dma_start(out=outT.ap()[ts(ot, 128), :], in_=o_sb)

    nc.compile()
    return nc


_CACHE = {}


def _get_nc():
    if "nc" not in _CACHE:
        _CACHE["nc"] = build_kernel()
    return _CACHE["nc"]


def make_inputs(positions, hidden_states, Wqkv, Wo):
    """Host-side shard prep. Returns per-core input maps."""
    bf = ml_dtypes.bfloat16
    positions = np.asarray(positions)
    hidden_states = np.asarray(hidden_states, dtype=np.float32)
    Wqkv = np.asarray(Wqkv, dtype=np.float32)
    Wo = np.asarray(Wo, dtype=np.float32)

    hT = np.ascontiguousarray(hidden_states.astype(bf).T)  # [HID, T]

    half = D // 2
    inv_freq = (
        1.0 / (np.float32(ROPE_THETA) ** (np.arange(0, half, dtype=np.float32) / np.float32(half)))
    ).astype(np.float32)
    freqs = positions.astype(np.float32)[:, None] * inv_freq[None, :]  # [T, 64]
    cos64 = np.cos(freqs).astype(np.float16).T  # [64, T]
    sin64 = np.sin(freqs).astype(np.float16).T
    # duplicated cos rows; sign-folded sin rows (see rope() in build_kernel)
    cosT = np.ascontiguousarray(np.concatenate([cos64, cos64], axis=0))
    sinT = np.ascontiguousarray(np.concatenate([-sin64, sin64], axis=0))

    q_size = NH * D
    kv_off = q_size + NKV * D
    in_maps = []
    for c in range(NCORES):
        qcols = Wqkv[:, 512 * c: 512 * (c + 1)]
        kcol = Wqkv[:, q_size + 128 * c: q_size + 128 * (c + 1)]
        vcol = Wqkv[:, kv_off + 128 * c: kv_off + 128 * (c + 1)]
        shard = np.concatenate([qcols, kcol, vcol], axis=1).astype(bf)  # [HID, 768]
        wq_dev = np.ascontiguousarray(
            shard.reshape(HO, 128, 6, 128).transpose(2, 1, 0, 3)
        )  # [6, 128, HO, 128]
        wo_shard = Wo[512 * c: 512 * (c + 1), :].astype(bf)  # [512, HID]
        wo_dev = np.ascontiguousarray(
            wo_shard.reshape(NHL, 128, HID).transpose(1, 0, 2)
        )  # [128, NHL, HID]
        in_maps.append(
            {
                "hT": hT,
                "wqkv": wq_dev,
                "wo": wo_dev,
                "cosT": cosT,
                "sinT": sinT,
            }
        )
    return in_maps


def kernel(positions, hidden_states, Wqkv, Wo):
    in_maps = make_inputs(positions, hidden_states, Wqkv, Wo)
    res = run_bass_kernel_spmd(_get_nc(), in_maps, list(range(NCORES)))
    acc = res.results[0]["outT"].copy()
    for c in range(1, NCORES):
        acc += res.results[c]["outT"]
    return np.ascontiguousarray(acc.T)
